# revision 9
# baseline (speedup 1.0000x reference)
"""GAE (advantage + return) reverse affine scan on 8 TRN2 NeuronCores.

Math: the reference's reversed lax.scan decomposes into two independent
first-order affine recurrences over t (run from T-1 down to 0):

    delta[i] = r[i] - v[i] + GAMMA * m[i] * v[i+1]          (pointwise)
    adv[i]   = delta[i] + (GAMMA*TAU*m[i]) * adv[i+1]        (affine scan)
    ret[i]   = (r[i] + GAMMA*(1-m[i])*nv[i]) + (GAMMA*m[i]) * ret[i+1]

Sharding: T is split into 8 contiguous chunks (one per core); each chunk is
laid out [128 partitions, F] with each partition owning a contiguous run of
F elements. Per-lane reverse scans run on the hardware tensor_tensor_scan
instruction via negative-stride access patterns. Carries across lanes/cores
use the affine composite (A, B) of each lane: tiny TensorE transposes + tiny
scans + one 16-float AllGather, then a second scan pass with corrected
initial carries.
"""

import numpy as np

GAMMA = 0.99
TAU = 0.95
P = 128
NCORES = 8

RET_SCAN_ENGINE = "vector"  # gpsimd rejected by walrus ISA check on TRN2

_graph_cache = {}


def _build_graph(F):
    import concourse.tile as tile
    from concourse import bacc, mybir

    f32 = mybir.dt.float32
    L = P * F
    nc = bacc.Bacc(
        "TRN2", target_bir_lowering=False, debug=False, num_devices=NCORES
    )

    r_ext = nc.declare_dram_parameter("rewards", [L, 1], f32, isOutput=False)
    v_ext = nc.declare_dram_parameter("values", [L + 1, 1], f32, isOutput=False)
    nv_ext = nc.declare_dram_parameter("next_values", [L, 1], f32, isOutput=False)
    m_ext = nc.declare_dram_parameter("masks", [L, 1], f32, isOutput=False)
    vb_ext = nc.declare_dram_parameter("vb", [P, 1], f32, isOutput=False)
    gt_ext = nc.declare_dram_parameter("mask_gt", [2, NCORES], f32, isOutput=False)
    le_ext = nc.declare_dram_parameter("mask_le", [2, NCORES], f32, isOutput=False)
    adv_ext = nc.declare_dram_parameter("adv", [L, 1], f32, isOutput=True)
    ret_ext = nc.declare_dram_parameter("ret", [L, 1], f32, isOutput=True)

    mult = mybir.AluOpType.mult
    add = mybir.AluOpType.add
    sub = mybir.AluOpType.subtract
    bypass = mybir.AluOpType.bypass
    Copy = mybir.ActivationFunctionType.Copy

    c_adv = GAMMA * TAU
    c_ret = GAMMA
    A_adv_F = float(np.float32(c_adv) ** F)  # may underflow to 0.0: correct
    A_ret_F = float(np.float32(c_ret) ** F)

    ret_eng_name = RET_SCAN_ENGINE

    with tile.TileContext(nc) as tc:
        ret_eng = nc.gpsimd if ret_eng_name == "gpsimd" else nc.vector
        with (
            tc.tile_pool(name="io", bufs=4) as io_pool,
            tc.tile_pool(name="ab", bufs=4) as ab_pool,
            tc.tile_pool(name="tmp", bufs=2) as tmp_pool,
            tc.tile_pool(name="small", bufs=1) as small,
            tc.tile_pool(name="dram", bufs=1, space="DRAM") as dram_pool,
        ):
            # ---- DMA in -------------------------------------------------
            m_t = io_pool.tile([P, F], f32, tag="io")
            nc.sync.dma_start(m_t[:], m_ext.rearrange("(p f) o -> p (f o)", p=P))
            r_t = io_pool.tile([P, F], f32, tag="io")
            nc.sync.dma_start(r_t[:], r_ext.rearrange("(p f) o -> p (f o)", p=P))
            v_t = io_pool.tile([P, F], f32, tag="io")
            nc.sync.dma_start(v_t[:], v_ext[0:L, :].rearrange("(p f) o -> p (f o)", p=P))
            nv_t = io_pool.tile([P, F], f32, tag="io")
            nc.sync.dma_start(nv_t[:], nv_ext.rearrange("(p f) o -> p (f o)", p=P))
            vb_t = small.tile([P, 1], f32)
            nc.gpsimd.dma_start(vb_t[:], vb_ext[:])
            gtile = small.tile([2, NCORES], f32)
            nc.gpsimd.dma_start(gtile[:], gt_ext[:])
            ltile = small.tile([2, NCORES], f32)
            nc.gpsimd.dma_start(ltile[:], le_ext[:])

            # ---- prep: a/b coefficient tensors --------------------------
            a_adv = ab_pool.tile([P, F], f32, tag="ab")
            nc.scalar.activation(a_adv[:], m_t[:], Copy, scale=c_adv)
            a_ret = ab_pool.tile([P, F], f32, tag="ab")
            nc.scalar.activation(a_ret[:], m_t[:], Copy, scale=c_ret)

            # allm[p] = all masks in lane p are 1. Sum the 0/1 masks on the
            # Scalar engine (integer partial sums are exact in fp32), then
            # threshold. The copy's full-width output goes to a scratch tile.
            msum = small.tile([P, 1], f32)
            mf = tmp_pool.tile([P, F], f32, tag="tmp")
            nc.scalar.activation(mf[:], m_t[:], Copy, scale=1.0, accum_out=msum[:])
            allm = small.tile([P, 1], f32)
            nc.vector.tensor_scalar(
                allm[:], msum[:], float(F) - 0.5, 0.0, mybir.AluOpType.is_ge, bypass
            )

            # delta = (r - v) + a_ret * v_shifted
            s1 = tmp_pool.tile([P, F], f32, tag="tmp")
            nc.vector.tensor_tensor(s1[:, 0 : F - 1], a_ret[:, 0 : F - 1], v_t[:, 1:F], mult)
            nc.vector.tensor_tensor(s1[:, F - 1 : F], a_ret[:, F - 1 : F], vb_t[:], mult)
            s2 = tmp_pool.tile([P, F], f32, tag="tmp")
            nc.vector.tensor_tensor(s2[:], r_t[:], v_t[:], sub)
            delta = ab_pool.tile([P, F], f32, tag="ab")
            nc.vector.tensor_tensor(delta[:], s1[:], s2[:], add)

            # b_ret = (GAMMA*nv + r) - a_ret*nv
            s3 = tmp_pool.tile([P, F], f32, tag="tmp")
            nc.vector.tensor_tensor(s3[:], a_ret[:], nv_t[:], mult)
            s4 = tmp_pool.tile([P, F], f32, tag="tmp")
            nc.vector.scalar_tensor_tensor(s4[:], nv_t[:], c_ret, r_t[:], mult, add)
            b_ret = ab_pool.tile([P, F], f32, tag="ab")
            nc.vector.tensor_tensor(b_ret[:], s4[:], s3[:], sub)

            # ---- phase 1: per-lane reverse scans with zero carry --------
            y0a = tmp_pool.tile([P, F], f32, tag="tmp")
            nc.vector.tensor_tensor_scan(
                y0a[:, ::-1], a_adv[:, ::-1], delta[:, ::-1], 0.0, mult, add
            )
            y0r = tmp_pool.tile([P, F], f32, tag="tmp")
            ret_eng.tensor_tensor_scan(
                y0r[:, ::-1], a_ret[:, ::-1], b_ret[:, ::-1], 0.0, mult, add
            )

            # ---- composites per lane: A = c^F*allm, B = y0[:, 0] --------
            acols = small.tile([P, 2], f32)
            nc.vector.tensor_scalar(acols[:, 0:1], allm[:], A_adv_F, 0.0, mult, bypass)
            nc.vector.tensor_scalar(acols[:, 1:2], allm[:], A_ret_F, 0.0, mult, bypass)
            bcols = small.tile([P, 2], f32)
            nc.vector.tensor_copy(bcols[:, 0:1], y0a[:, 0:1])
            nc.vector.tensor_copy(bcols[:, 1:2], y0r[:, 0:1])

            # tiny transposes via DRAM bounce + AP swap
            dA = dram_pool.tile([P, 2], f32)
            nc.gpsimd.dma_start(dA[:], acols[:])
            arow = small.tile([2, P], f32)
            nc.gpsimd.dma_start(arow[:], dA[:].rearrange("a b -> b a"))
            dB = dram_pool.tile([P, 2], f32)
            nc.gpsimd.dma_start(dB[:], bcols[:])
            brow = small.tile([2, P], f32)
            nc.gpsimd.dma_start(brow[:], dB[:].rearrange("a b -> b a"))

            # core composite: compose lanes 127..0 applied to 0 / product of A
            bcomp = small.tile([2, P], f32)
            nc.vector.tensor_tensor_scan(
                bcomp[:, ::-1], arow[:, ::-1], brow[:, ::-1], 0.0, mult, add
            )
            ones2 = small.tile([2, P], f32)
            nc.vector.memset(ones2[:], 1.0)
            acomp = small.tile([2, P], f32)
            nc.vector.tensor_tensor_scan(
                acomp[:, ::-1], arow[:, ::-1], ones2[:, ::-1], 1.0, mult, mult
            )

            # ---- cross-core exchange: AllGather of (A_core, B_core) -----
            ccin_t = small.tile([2, 2], f32)
            nc.vector.tensor_copy(ccin_t[:, 0:1], acomp[:, 0:1])
            nc.vector.tensor_copy(ccin_t[:, 1:2], bcomp[:, 0:1])
            cc_in = dram_pool.tile([2, 2], f32)
            cc_out = dram_pool.tile([2 * NCORES, 2], f32, addr_space="Shared")
            nc.gpsimd.dma_start(cc_in[:], ccin_t[:])
            nc.gpsimd.collective_compute(
                "AllGather",
                bypass,
                replica_groups=[list(range(NCORES))],
                ins=[cc_in[:].opt()],
                outs=[cc_out[:].opt()],
            )
            Aall = small.tile([2, NCORES], f32)
            nc.gpsimd.dma_start(
                Aall[:], cc_out[:].rearrange("(j r) c -> r j c", r=2)[:, :, 0:1]
            )
            Ball = small.tile([2, NCORES], f32)
            nc.gpsimd.dma_start(
                Ball[:], cc_out[:].rearrange("(j r) c -> r j c", r=2)[:, :, 1:2]
            )

            # blend to identity for cores <= self, then compose 7..0
            tA = small.tile([2, NCORES], f32)
            nc.vector.tensor_tensor(tA[:], Aall[:], gtile[:], mult)
            tA2 = small.tile([2, NCORES], f32)
            nc.vector.tensor_tensor(tA2[:], tA[:], ltile[:], add)
            tB = small.tile([2, NCORES], f32)
            nc.vector.tensor_tensor(tB[:], Ball[:], gtile[:], mult)
            ccomp = small.tile([2, NCORES], f32)
            nc.vector.tensor_tensor_scan(
                ccomp[:, ::-1], tA2[:, ::-1], tB[:, ::-1], 0.0, mult, add
            )

            # lane-level carries: scan lanes 127..0 with core carry as init
            ls = small.tile([2, P], f32)
            nc.vector.tensor_tensor_scan(
                ls[:, ::-1], arow[:, ::-1], brow[:, ::-1], ccomp[:, 0:1], mult, add
            )
            carry_row = small.tile([2, P], f32)
            nc.vector.tensor_copy(carry_row[:, 0 : P - 1], ls[:, 1:P])
            nc.vector.tensor_copy(carry_row[:, P - 1 : P], ccomp[:, 0:1])

            dC = dram_pool.tile([2, P], f32)
            nc.gpsimd.dma_start(dC[:], carry_row[:])
            carr = small.tile([P, 2], f32)
            nc.gpsimd.dma_start(carr[:], dC[:].rearrange("a b -> b a"))

            # ---- phase 3: rescan with corrected carries -----------------
            yadv = io_pool.tile([P, F], f32, tag="io")
            nc.vector.tensor_tensor_scan(
                yadv[:, ::-1], a_adv[:, ::-1], delta[:, ::-1], carr[:, 0:1], mult, add
            )
            yret = io_pool.tile([P, F], f32, tag="io")
            ret_eng.tensor_tensor_scan(
                yret[:, ::-1], a_ret[:, ::-1], b_ret[:, ::-1], carr[:, 1:2], mult, add
            )

            nc.sync.dma_start(adv_ext.rearrange("(p f) o -> p (f o)", p=P), yadv[:])
            nc.sync.dma_start(ret_ext.rearrange("(p f) o -> p (f o)", p=P), yret[:])

    nc.compile()
    return nc


def get_graph(F):
    key = (F, RET_SCAN_ENGINE)
    if key not in _graph_cache:
        _graph_cache[key] = _build_graph(F)
    return _graph_cache[key]


def make_in_maps(rewards, values, next_values, masks):
    T = rewards.shape[0]
    L = T // NCORES
    F = L // P
    r = np.ascontiguousarray(rewards, dtype=np.float32).reshape(T, 1)
    nv = np.ascontiguousarray(next_values, dtype=np.float32).reshape(T, 1)
    m = np.ascontiguousarray(masks).astype(np.float32).reshape(T, 1)
    vpad = np.empty((T + 1, 1), dtype=np.float32)
    vpad[:T] = np.asarray(values, dtype=np.float32).reshape(T, 1)
    vpad[T] = 0.0
    in_maps = []
    for k in range(NCORES):
        base = k * L
        gt = np.zeros((2, NCORES), dtype=np.float32)
        gt[:, k + 1 :] = 1.0
        vb = vpad[base + F : base + L + F : F, :][:P].copy()
        in_maps.append(
            {
                "rewards": r[base : base + L],
                "values": vpad[base : base + L + 1],
                "next_values": nv[base : base + L],
                "masks": m[base : base + L],
                "vb": vb,
                "mask_gt": gt,
                "mask_le": np.float32(1.0) - gt,
            }
        )
    return in_maps, L, F


def kernel(rewards, values, next_values, masks):
    from concourse.bass_utils import run_bass_kernel_spmd

    in_maps, L, F = make_in_maps(rewards, values, next_values, masks)
    nc = get_graph(F)
    res = run_bass_kernel_spmd(nc, in_maps, core_ids=list(range(NCORES))).results
    adv = np.concatenate([res[k]["adv"] for k in range(NCORES)], axis=0)
    ret = np.concatenate([res[k]["ret"] for k in range(NCORES)], axis=0)
    return adv, ret
